# revision 8
# baseline (speedup 1.0000x reference)
"""Expert-mixture (top-1 MoE) Trainium2 kernel, expert-parallel across 8 cores.

Strategy:
  - Host computes the router (x @ Wr + br, argmax) and dispatches tokens:
    all tokens routed to expert e are gathered, transposed, and padded to a
    fixed capacity CAP=2048, forming core e's shard. Tokens beyond CAP
    (194 rows for the reference seed) are computed on host.
  - Core e computes hT = relu(W1[e].T @ xT + b1[e]) then outT = W2[e].T @ h
    entirely on-device: bf16 operands (FWL weight loads, half the DMA bytes
    of fp32r at the same 1 cycle/row PE rate), f32 PSUM accumulation.
  - Host scatters each expert's rows back into the full [B, C] output and
    adds b2[topic] (the bias add commutes with the gather).

Device schedule (per core, SPMD):
  - 4 token blocks of 512 columns; GEMM1 runs in 4 groups of 4 H-chunks
    (4 PSUM banks per group, 6-bank rotation), k-outer within a group.
  - GEMM2 rounds are issued one GEMM1-group late so the ACT-engine relu of a
    group fully overlaps the next group's matmuls. For blocks 0..2 the four
    chunk-matmuls of a round run concurrently in four PE column groups
    (tile_position=(0,32j)); the four column-group partials are summed by a
    DVE chain straight out of PSUM. The last block uses a single plain
    accumulation group (no partial-sum chain in the kernel tail) and its
    last GEMM1 group runs mi-outer so the final relu is off the critical
    path.
  - DMA issues are spread across three engine queues (w1 on SP, xt on DVE,
    the rest on ACT) because each DMA_DIRECT2D occupies its queue ~650ns;
    a single queue would starve the first GEMM1 group.
  - A few bf16 warmup matmuls bridge the PE from the NEFF preamble to the
    first data-dependent matmul so the HAM clock gate sees sustained
    activity (cold PE runs at 1.2 GHz vs 2.4 GHz warm).

The builder is exec'd from a string with a fixed pseudo-filename so the
emitted BIR (which embeds source file/line debug info) is byte-identical no
matter where this file lives — keeping the NEFF compile cache warm across
directories.
"""

import numpy as np

import concourse.mybir as mybir
import concourse.tile as tile
from concourse import bacc
from concourse.bass_utils import run_bass_kernel_spmd

B, D, H, E, C = 16384, 1024, 2048, 8, 3
N_CORES = 8
P = 128
KD = D // P    # 8 contraction chunks for GEMM1
MH = H // P    # 16 H chunks
TB = 512       # token block (matmul moving dim; one f32 PSUM bank)
CAP = 2048     # per-expert token capacity (== B/E); overflow rows (1.2% of
               # tokens for the reference seed) are computed on host

WARMUP_MMS = 5   # bf16 N=512 matmuls bridging preamble -> first real matmul

_nc_cache: dict = {}

_BUILDER_SRC = '''
def _build(cap, reps, warmup_mms, mybir, tile, bacc):
    B, D, H, E, C = 16384, 1024, 2048, 8, 3
    N_CORES, P = 8, 128
    KD, MH, TB = D // P, H // P, 512
    GSZ = 8            # H-chunks per GEMM1 group
    NG = MH // GSZ     # groups per block
    NBLK = cap // TB
    assert cap % TB == 0

    nc = bacc.Bacc("TRN2", target_bir_lowering=False, debug=False,
                   num_devices=N_CORES)
    f32 = mybir.dt.float32
    bf16 = mybir.dt.bfloat16
    Add = mybir.AluOpType.add
    Relu = mybir.ActivationFunctionType.Relu

    xt = nc.dram_tensor("xt", [D, cap], bf16, kind="ExternalInput").ap()
    w1 = nc.dram_tensor("w1", [D, H], bf16, kind="ExternalInput").ap()
    b1t = nc.dram_tensor("b1t", [P, MH], f32, kind="ExternalInput").ap()
    w2t = nc.dram_tensor("w2t", [P, MH * C], bf16,
                         kind="ExternalInput").ap()
    ot = nc.dram_tensor("ot", [C, cap], f32, kind="ExternalOutput").ap()

    with tile.TileContext(nc) as tc:
        with (
            tc.tile_pool(name="w1p", bufs=1) as w1p,
            tc.tile_pool(name="xtp", bufs=1) as xtp,
            tc.tile_pool(name="cst", bufs=1) as cst,
            tc.tile_pool(name="htp", bufs=1) as htp,
            tc.tile_pool(name="o2p", bufs=1) as o2p,
            tc.tile_pool(name="smp", bufs=1) as smp,
            tc.tile_pool(name="ps", bufs=1, space="PSUM") as psp,
        ):
            def body(_iv=None):
                w1_sb = w1p.tile([P, KD * H], bf16, tag="w1", name="w1_sb")
                xt_sbs = [xtp.tile([P, KD * TB], bf16, tag="xt", bufs=NBLK,
                                   name="xt_%d" % t) for t in range(NBLK)]
                b1_sb = cst.tile([P, MH], f32, tag="b1", name="b1_sb")
                w2_sb = cst.tile([P, MH * C], bf16, tag="w2", name="w2_sb")
                o2_sb = o2p.tile([C, cap], f32, tag="o2", name="o2_sb")
                wu = cst.tile([P, TB], bf16, tag="wu", name="wu")
                nc.vector.memset(wu[:], 0.0)

                # PE warmup: keep the PE busy from the end of the NEFF
                # preamble until block-0 data lands so the HAM clock gate
                # sees sustained activity going into the real stream.
                wps = psp.tile([P, TB], f32, tag="ps2", bufs=2, name="wps")
                for _ in range(warmup_mms):
                    nc.tensor.matmul(wps[:], wu[:, 0:P], wu[:],
                                     start=True, stop=True)

                # ---- DMA issues, spread across three engine queues ----
                w1v = w1.rearrange("(k p) h -> p k h", k=KD)
                w1s = w1_sb[:].rearrange("p (k h) -> p k h", k=KD)
                xtv = xt.rearrange("(k p) n -> p k n", k=KD)

                def xt_dst(t):
                    return xt_sbs[t][:].rearrange("p (k n) -> p k n", k=KD)

                # ACT queue: biases + W2 (tiny, needed from the first
                # relu). Everything else goes on the SP queue in strict
                # consumption order: the DMA runtime shards every transfer
                # across all 16 engines and time-slices CONCURRENT
                # transfers fairly, so spreading issues over multiple
                # queues makes critical early bytes finish late. A single
                # ordered queue keeps the first GEMM1 group fed.
                nc.scalar.dma_start(b1_sb[:], b1t[:])
                nc.scalar.dma_start(w2_sb[:], w2t[:])

                Q = GSZ * P  # 1024 weight columns per group
                nc.sync.dma_start(xt_dst(0)[:, 0:1, :], xtv[:, 0:1, 0:TB])
                nc.sync.dma_start(w1s[:, 0:1, 0:2 * P], w1v[:, 0:1, 0:2 * P])
                nc.sync.dma_start(w1s[:, 0:1, 2 * P:Q], w1v[:, 0:1, 2 * P:Q])
                for k in range(1, KD):
                    nc.sync.dma_start(w1s[:, k:k + 1, 0:Q],
                                      w1v[:, k:k + 1, 0:Q])
                    nc.sync.dma_start(xt_dst(0)[:, k:k + 1, :],
                                      xtv[:, k:k + 1, 0:TB])
                for k in range(KD):
                    nc.sync.dma_start(w1s[:, k:k + 1, Q:H],
                                      w1v[:, k:k + 1, Q:H])
                for t in range(1, NBLK):
                    nc.sync.dma_start(xt_dst(t)[:, :, :],
                                      xtv[:, :, t * TB:(t + 1) * TB])

                # ---- main schedule ----
                ps2_t = [None] * NBLK
                ht_tiles = {}

                def emit_g1(t, g, mi_outer):
                    ps_g = [psp.tile([P, TB], f32, tag="ps1", bufs=6,
                                     name="ps1_%d_%d_%d" % (t, g, mi))
                            for mi in range(GSZ)]

                    def mm(k, mi):
                        m = g * GSZ + mi
                        nc.tensor.matmul(
                            ps_g[mi][:, :],
                            w1_sb[:, k * H + m * P:k * H + (m + 1) * P],
                            xt_sbs[t][:, k * TB:(k + 1) * TB],
                            start=(k == 0),
                            stop=(k == KD - 1),
                        )

                    def relu(mi):
                        m = g * GSZ + mi
                        ht = htp.tile([P, TB], bf16, tag="ht%d" % m, bufs=2,
                                      name="ht_%d_%d" % (t, m))
                        ht_tiles[(t, m)] = ht
                        nc.scalar.activation(ht[:], ps_g[mi][:], Relu,
                                             bias=b1_sb[:, m:m + 1])

                    if mi_outer:
                        for mi in range(GSZ):
                            for k in range(KD):
                                mm(k, mi)
                            relu(mi)
                    else:
                        for k in range(KD):
                            for mi in range(GSZ):
                                mm(k, mi)
                        for mi in range(GSZ):
                            relu(mi)

                def emit_g2(t, g):
                    if g == 0:
                        ps2_t[t] = psp.tile([P, TB], f32, tag="ps2", bufs=2,
                                            name="ps2_%d" % t)
                    ps2 = ps2_t[t]
                    ms = [g * GSZ + j for j in range(GSZ)]
                    if (t, g) == (NBLK - 1, NG - 1):
                        # The final batch: issue the last H-chunk first --
                        # its relu is the last to finish, and putting it
                        # first hides that wait under the other chunks.
                        ms = ms[-1:] + ms[:-1]
                    for i, m in enumerate(ms):
                        ht = ht_tiles.pop((t, m))
                        nc.tensor.matmul(
                            ps2[0:C, :], w2_sb[:, m * C:(m + 1) * C],
                            ht[:], start=(m == 0),
                            stop=(g == NG - 1 and i == GSZ - 1),
                            skip_group_check=True)
                    if g == NG - 1:
                        toff = t * TB
                        osl = o2_sb[:, toff:toff + TB]
                        nc.vector.tensor_copy(osl, ps2[0:C, :])
                        nc.scalar.dma_start(ot[:, toff:toff + TB], osl)

                pairs = [(t, g) for t in range(NBLK) for g in range(NG)]
                for i, (t, g) in enumerate(pairs):
                    emit_g1(t, g, mi_outer=(i == len(pairs) - 1))
                    if i >= 1:
                        emit_g2(*pairs[i - 1])
                emit_g2(*pairs[-1])

            if reps == 1:
                body()
            else:
                hints = (mybir.EngineType.PE, mybir.EngineType.SP,
                         mybir.EngineType.Activation, mybir.EngineType.DVE)
                with tc.For_i(0, reps, 1, hint_engines=hints) as iv:
                    body(iv)

    nc.compile()
    return nc
'''

_builder_ns: dict = {}
exec(compile(_BUILDER_SRC, "<moe_builder>", "exec"), _builder_ns)


def build_nc(cap: int, reps: int = 1):
    """Build + compile the SPMD program. reps>1 wraps the body in a device
    loop (for steady-state timing); data loads stay inside the loop so each
    iteration models one cold kernel execution."""
    return _builder_ns["_build"](cap, reps, WARMUP_MMS, mybir, tile, bacc)


def _get_nc(cap: int):
    if cap not in _nc_cache:
        _nc_cache[cap] = build_nc(cap)
    return _nc_cache[cap]


def _expert_mlp_host(xr, W1e, b1e, W2e, b2e):
    h = np.maximum(xr.astype(np.float32) @ W1e + b1e, 0.0)
    return h @ W2e + b2e


def _to_mm(a: np.ndarray) -> np.ndarray:
    """Convert f32 host data to the matmul storage dtype (bf16,
    round-to-nearest-even)."""
    import ml_dtypes
    return np.ascontiguousarray(a, dtype=np.float32).astype(ml_dtypes.bfloat16)


def make_in_maps(x, W1, b1, W2, idx, cap):
    import ml_dtypes
    in_maps = []
    for e in range(E):
        ie = idx[e][:cap]
        xtc = np.zeros((D, cap), dtype=ml_dtypes.bfloat16)
        xtc[:, :len(ie)] = _to_mm(x[ie].T)
        in_maps.append({
            "xt": xtc,
            "w1": _to_mm(W1[e]),
            "b1t": np.ascontiguousarray(b1[e].reshape(MH, P).T),
            "w2t": _to_mm(
                W2[e].reshape(MH, P, C).transpose(1, 0, 2).reshape(P, MH * C)),
        })
    return in_maps


def kernel(x, Wr, br, W1, b1, W2, b2):
    x = np.asarray(x, dtype=np.float32)
    Wr = np.asarray(Wr, dtype=np.float32)
    br = np.asarray(br, dtype=np.float32)
    W1 = np.asarray(W1, dtype=np.float32)
    b1 = np.asarray(b1, dtype=np.float32)
    W2 = np.asarray(W2, dtype=np.float32)
    b2 = np.asarray(b2, dtype=np.float32)

    # Router on host: this decides the (expert-parallel) sharding. Use CPU
    # jax for the logits so near-tie argmax decisions round exactly like the
    # reference's jnp expression; fall back to numpy if no CPU backend.
    try:
        import jax
        import jax.numpy as jnp
        with jax.default_device(jax.devices("cpu")[0]):
            logits = np.asarray(jnp.asarray(x) @ jnp.asarray(Wr)
                                + jnp.asarray(br))
    except Exception:
        logits = x @ Wr + br
    topics = np.argmax(logits, axis=1)

    idx = [np.flatnonzero(topics == e) for e in range(E)]
    cap = CAP
    in_maps = make_in_maps(x, W1, b1, W2, idx, cap)
    nc = _get_nc(cap)
    res = run_bass_kernel_spmd(nc, in_maps, core_ids=list(range(N_CORES)))

    out = np.empty((B, C), dtype=np.float32)
    for e in range(E):
        ie = idx[e][:cap]
        out[ie] = res.results[e]["ot"][:, :len(ie)].T + b2[e]
        if len(idx[e]) > cap:
            ov = idx[e][cap:]
            out[ov] = _expert_mlp_host(x[ov], W1[e], b1[e], W2[e], b2[e])
    return out


# revision 9
# speedup vs baseline: 1.0055x; 1.0055x over previous
"""Expert-mixture (top-1 MoE) Trainium2 kernel, expert-parallel across 8 cores.

Strategy:
  - Host computes the router (x @ Wr + br, argmax) and dispatches tokens:
    all tokens routed to expert e are gathered, transposed, and padded to a
    fixed capacity CAP=2048, forming core e's shard. Tokens beyond CAP
    (194 rows for the reference seed) are computed on host.
  - Core e computes hT = relu(W1[e].T @ xT + b1[e]) then outT = W2[e].T @ h
    entirely on-device: bf16 operands (FWL weight loads, half the DMA bytes
    of fp32r at the same 1 cycle/row PE rate), f32 PSUM accumulation.
  - Host scatters each expert's rows back into the full [B, C] output and
    adds b2[topic] (the bias add commutes with the gather).

Device schedule (per core, SPMD):
  - 4 token blocks of 512 columns; GEMM1 runs in 4 groups of 4 H-chunks
    (4 PSUM banks per group, 6-bank rotation), k-outer within a group.
  - GEMM2 rounds are issued one GEMM1-group late so the ACT-engine relu of a
    group fully overlaps the next group's matmuls. For blocks 0..2 the four
    chunk-matmuls of a round run concurrently in four PE column groups
    (tile_position=(0,32j)); the four column-group partials are summed by a
    DVE chain straight out of PSUM. The last block uses a single plain
    accumulation group (no partial-sum chain in the kernel tail) and its
    last GEMM1 group runs mi-outer so the final relu is off the critical
    path.
  - DMA issues are spread across three engine queues (w1 on SP, xt on DVE,
    the rest on ACT) because each DMA_DIRECT2D occupies its queue ~650ns;
    a single queue would starve the first GEMM1 group.
  - A few bf16 warmup matmuls bridge the PE from the NEFF preamble to the
    first data-dependent matmul so the HAM clock gate sees sustained
    activity (cold PE runs at 1.2 GHz vs 2.4 GHz warm).

The builder is exec'd from a string with a fixed pseudo-filename so the
emitted BIR (which embeds source file/line debug info) is byte-identical no
matter where this file lives — keeping the NEFF compile cache warm across
directories.
"""

import numpy as np

import concourse.mybir as mybir
import concourse.tile as tile
from concourse import bacc
from concourse.bass_utils import run_bass_kernel_spmd

B, D, H, E, C = 16384, 1024, 2048, 8, 3
N_CORES = 8
P = 128
KD = D // P    # 8 contraction chunks for GEMM1
MH = H // P    # 16 H chunks
TB = 512       # token block (matmul moving dim; one f32 PSUM bank)
CAP = 2048     # per-expert token capacity (== B/E); overflow rows (1.2% of
               # tokens for the reference seed) are computed on host

WARMUP_MMS = 8   # bf16 N=512 matmuls bridging preamble -> first real matmul

_nc_cache: dict = {}

_BUILDER_SRC = '''
def _build(cap, reps, warmup_mms, mybir, tile, bacc):
    B, D, H, E, C = 16384, 1024, 2048, 8, 3
    N_CORES, P = 8, 128
    KD, MH, TB = D // P, H // P, 512
    GSZ = 8            # H-chunks per GEMM1 group
    NG = MH // GSZ     # groups per block
    NBLK = cap // TB
    assert cap % TB == 0

    nc = bacc.Bacc("TRN2", target_bir_lowering=False, debug=False,
                   num_devices=N_CORES)
    f32 = mybir.dt.float32
    bf16 = mybir.dt.bfloat16
    Add = mybir.AluOpType.add
    Relu = mybir.ActivationFunctionType.Relu

    xt = nc.dram_tensor("xt", [D, cap], bf16, kind="ExternalInput").ap()
    w1 = nc.dram_tensor("w1", [D, H], bf16, kind="ExternalInput").ap()
    b1t = nc.dram_tensor("b1t", [P, MH], f32, kind="ExternalInput").ap()
    w2t = nc.dram_tensor("w2t", [P, MH * C], bf16,
                         kind="ExternalInput").ap()
    ot = nc.dram_tensor("ot", [C, cap], f32, kind="ExternalOutput").ap()

    with tile.TileContext(nc) as tc:
        with (
            tc.tile_pool(name="w1p", bufs=1) as w1p,
            tc.tile_pool(name="xtp", bufs=1) as xtp,
            tc.tile_pool(name="cst", bufs=1) as cst,
            tc.tile_pool(name="htp", bufs=1) as htp,
            tc.tile_pool(name="o2p", bufs=1) as o2p,
            tc.tile_pool(name="smp", bufs=1) as smp,
            tc.tile_pool(name="ps", bufs=1, space="PSUM") as psp,
        ):
            def body(_iv=None):
                w1_sb = w1p.tile([P, KD * H], bf16, tag="w1", name="w1_sb")
                xt_sbs = [xtp.tile([P, KD * TB], bf16, tag="xt", bufs=NBLK,
                                   name="xt_%d" % t) for t in range(NBLK)]
                b1_sb = cst.tile([P, MH], f32, tag="b1", name="b1_sb")
                w2_sb = cst.tile([P, MH * C], bf16, tag="w2", name="w2_sb")
                o2_sb = o2p.tile([C, cap], f32, tag="o2", name="o2_sb")
                wu = cst.tile([P, TB], bf16, tag="wu", name="wu")
                nc.vector.memset(wu[:], 0.0)

                # PE warmup: keep the PE busy from the end of the NEFF
                # preamble until block-0 data lands so the HAM clock gate
                # sees sustained activity going into the real stream.
                wps = psp.tile([P, TB], f32, tag="ps2", bufs=2, name="wps")
                for _ in range(warmup_mms):
                    nc.tensor.matmul(wps[:], wu[:, 0:P], wu[:],
                                     start=True, stop=True)

                # ---- DMA issues, spread across three engine queues ----
                w1v = w1.rearrange("(k p) h -> p k h", k=KD)
                w1s = w1_sb[:].rearrange("p (k h) -> p k h", k=KD)
                xtv = xt.rearrange("(k p) n -> p k n", k=KD)

                def xt_dst(t):
                    return xt_sbs[t][:].rearrange("p (k n) -> p k n", k=KD)

                # ACT queue: biases + W2 (tiny, needed from the first
                # relu). Everything else goes on the SP queue in strict
                # consumption order: the DMA runtime shards every transfer
                # across all 16 engines and time-slices CONCURRENT
                # transfers fairly, so spreading issues over multiple
                # queues makes critical early bytes finish late. A single
                # ordered queue keeps the first GEMM1 group fed.
                nc.scalar.dma_start(b1_sb[:], b1t[:])
                nc.scalar.dma_start(w2_sb[:], w2t[:])

                Q = GSZ * P  # 1024 weight columns per group
                nc.sync.dma_start(xt_dst(0)[:, 0:1, :], xtv[:, 0:1, 0:TB])
                nc.sync.dma_start(w1s[:, 0:1, 0:2 * P], w1v[:, 0:1, 0:2 * P])
                nc.sync.dma_start(w1s[:, 0:1, 2 * P:Q], w1v[:, 0:1, 2 * P:Q])
                for k in range(1, KD):
                    nc.sync.dma_start(w1s[:, k:k + 1, 0:Q],
                                      w1v[:, k:k + 1, 0:Q])
                    nc.sync.dma_start(xt_dst(0)[:, k:k + 1, :],
                                      xtv[:, k:k + 1, 0:TB])
                for k in range(KD):
                    nc.sync.dma_start(w1s[:, k:k + 1, Q:H],
                                      w1v[:, k:k + 1, Q:H])
                for t in range(1, NBLK):
                    nc.sync.dma_start(xt_dst(t)[:, :, :],
                                      xtv[:, :, t * TB:(t + 1) * TB])

                # ---- main schedule ----
                ps2_t = [None] * NBLK
                ht_tiles = {}

                def emit_g1(t, g, mi_outer):
                    ps_g = [psp.tile([P, TB], f32, tag="ps1", bufs=6,
                                     name="ps1_%d_%d_%d" % (t, g, mi))
                            for mi in range(GSZ)]

                    def mm(k, mi):
                        m = g * GSZ + mi
                        nc.tensor.matmul(
                            ps_g[mi][:, :],
                            w1_sb[:, k * H + m * P:k * H + (m + 1) * P],
                            xt_sbs[t][:, k * TB:(k + 1) * TB],
                            start=(k == 0),
                            stop=(k == KD - 1),
                        )

                    def relu(mi):
                        m = g * GSZ + mi
                        ht = htp.tile([P, TB], bf16, tag="ht%d" % m, bufs=2,
                                      name="ht_%d_%d" % (t, m))
                        ht_tiles[(t, m)] = ht
                        nc.scalar.activation(ht[:], ps_g[mi][:], Relu,
                                             bias=b1_sb[:, m:m + 1])

                    if mi_outer:
                        for mi in range(GSZ):
                            for k in range(KD):
                                mm(k, mi)
                            relu(mi)
                    else:
                        for k in range(KD):
                            for mi in range(GSZ):
                                mm(k, mi)
                        for mi in range(GSZ):
                            relu(mi)

                def emit_g2(t, g):
                    if g == 0:
                        ps2_t[t] = psp.tile([P, TB], f32, tag="ps2", bufs=2,
                                            name="ps2_%d" % t)
                    ps2 = ps2_t[t]
                    ms = [g * GSZ + j for j in range(GSZ)]
                    if (t, g) == (NBLK - 1, NG - 1):
                        # The final batch: issue the last H-chunk first --
                        # its relu is the last to finish, and putting it
                        # first hides that wait under the other chunks.
                        ms = ms[-1:] + ms[:-1]
                    for i, m in enumerate(ms):
                        ht = ht_tiles.pop((t, m))
                        nc.tensor.matmul(
                            ps2[0:C, :], w2_sb[:, m * C:(m + 1) * C],
                            ht[:], start=(m == 0),
                            stop=(g == NG - 1 and i == GSZ - 1),
                            skip_group_check=True)
                    if g == NG - 1:
                        toff = t * TB
                        osl = o2_sb[:, toff:toff + TB]
                        nc.vector.tensor_copy(osl, ps2[0:C, :])
                        nc.scalar.dma_start(ot[:, toff:toff + TB], osl)

                pairs = [(t, g) for t in range(NBLK) for g in range(NG)]
                for i, (t, g) in enumerate(pairs):
                    emit_g1(t, g, mi_outer=(i == len(pairs) - 1))
                    if i >= 1:
                        emit_g2(*pairs[i - 1])
                emit_g2(*pairs[-1])

            if reps == 1:
                body()
            else:
                hints = (mybir.EngineType.PE, mybir.EngineType.SP,
                         mybir.EngineType.Activation, mybir.EngineType.DVE)
                with tc.For_i(0, reps, 1, hint_engines=hints) as iv:
                    body(iv)

    nc.compile()
    return nc
'''

_builder_ns: dict = {}
exec(compile(_BUILDER_SRC, "<moe_builder>", "exec"), _builder_ns)


def build_nc(cap: int, reps: int = 1):
    """Build + compile the SPMD program. reps>1 wraps the body in a device
    loop (for steady-state timing); data loads stay inside the loop so each
    iteration models one cold kernel execution."""
    return _builder_ns["_build"](cap, reps, WARMUP_MMS, mybir, tile, bacc)


def _get_nc(cap: int):
    if cap not in _nc_cache:
        _nc_cache[cap] = build_nc(cap)
    return _nc_cache[cap]


def _expert_mlp_host(xr, W1e, b1e, W2e, b2e):
    h = np.maximum(xr.astype(np.float32) @ W1e + b1e, 0.0)
    return h @ W2e + b2e


def _to_mm(a: np.ndarray) -> np.ndarray:
    """Convert f32 host data to the matmul storage dtype (bf16,
    round-to-nearest-even)."""
    import ml_dtypes
    return np.ascontiguousarray(a, dtype=np.float32).astype(ml_dtypes.bfloat16)


def make_in_maps(x, W1, b1, W2, idx, cap):
    import ml_dtypes
    in_maps = []
    for e in range(E):
        ie = idx[e][:cap]
        xtc = np.zeros((D, cap), dtype=ml_dtypes.bfloat16)
        xtc[:, :len(ie)] = _to_mm(x[ie].T)
        in_maps.append({
            "xt": xtc,
            "w1": _to_mm(W1[e]),
            "b1t": np.ascontiguousarray(b1[e].reshape(MH, P).T),
            "w2t": _to_mm(
                W2[e].reshape(MH, P, C).transpose(1, 0, 2).reshape(P, MH * C)),
        })
    return in_maps


def kernel(x, Wr, br, W1, b1, W2, b2):
    x = np.asarray(x, dtype=np.float32)
    Wr = np.asarray(Wr, dtype=np.float32)
    br = np.asarray(br, dtype=np.float32)
    W1 = np.asarray(W1, dtype=np.float32)
    b1 = np.asarray(b1, dtype=np.float32)
    W2 = np.asarray(W2, dtype=np.float32)
    b2 = np.asarray(b2, dtype=np.float32)

    # Router on host: this decides the (expert-parallel) sharding. Use CPU
    # jax for the logits so near-tie argmax decisions round exactly like the
    # reference's jnp expression; fall back to numpy if no CPU backend.
    try:
        import jax
        import jax.numpy as jnp
        with jax.default_device(jax.devices("cpu")[0]):
            logits = np.asarray(jnp.asarray(x) @ jnp.asarray(Wr)
                                + jnp.asarray(br))
    except Exception:
        logits = x @ Wr + br
    topics = np.argmax(logits, axis=1)

    idx = [np.flatnonzero(topics == e) for e in range(E)]
    cap = CAP
    in_maps = make_in_maps(x, W1, b1, W2, idx, cap)
    nc = _get_nc(cap)
    res = run_bass_kernel_spmd(nc, in_maps, core_ids=list(range(N_CORES)))

    out = np.empty((B, C), dtype=np.float32)
    for e in range(E):
        ie = idx[e][:cap]
        out[ie] = res.results[e]["ot"][:, :len(ie)].T + b2[e]
        if len(idx[e]) > cap:
            ov = idx[e][cap:]
            out[ov] = _expert_mlp_host(x[ov], W1[e], b1[e], W2[e], b2[e])
    return out
